# revision 5
# baseline (speedup 1.0000x reference)
"""BGE-M3 sparse-embedding head (matvec + relu + scatter-max into (B, V))
as a Bass/Tile kernel on 8 Trainium2 NeuronCores.

Sharding: data-parallel over batch; each core computes 4 of 32 rows.

Per core:
  1. tw = relu(hidden @ w + b) streamed in 128-token tiles, computed with a
     fused scalar_tensor_tensor (multiply + free-dim sum) on the vector engine.
  2. Output rows are assembled DENSELY in SBUF and written out with plain
     DMAs (no data-dependent scatter DMAs): for each row, two exact one-hot
     factors route every singleton token's weight to its (partition, offset)
     cell of a (128, 1954) tile via PE matmuls:
        A_k[t, p] = (iota128 == id//1954) * tw[t]   (per-partition scalars)
        R_k[t, f] = (iota1954 == id%1954)
        dense += A_k^T @ R_k   (accumulated over the row's 8 token chunks)
     Distinct vocab ids hit distinct cells, so the sums are exact.
  3. Duplicate vocab ids within a row (a handful per row; the class structure
     is a pure function of input_ids, so the host computes it) are excluded
     from the dense path and resolved exactly by a small matmul that buckets
     each class's member weights into one PSUM row, a free-dim reduce_max,
     and ONE 128-index indirect DMA scatter per row (disjoint from the dense
     positions, so no ordering hazards beyond row-level sequencing).
Special tokens 0..3 are never routed, leaving zeros from the dense tile.
"""

import numpy as np

import concourse.bass as bass
import concourse.mybir as mybir
import concourse.tile as tile
from concourse.bass import IndirectOffsetOnAxis
from concourse.bass_utils import run_bass_kernel_spmd

V = 250002
NCORES = 8
B, L, H = 32, 1024, 1024
BS = B // NCORES            # batch rows per core
NT = BS * L                 # tokens per core
P = 128
CPR = L // P                # chunks per row (8)
NCHUNK = NT // P            # chunks per core (32)
W = 1954                    # dense row width per partition (128*1954 >= V)
MAXCLS = P                  # fixup classes per row (<=128)
MAXMEM = 8                  # members per duplicate class
F32 = mybir.dt.float32
I32 = mybir.dt.int32

_MAX_WAITS = 1


def _split_excess_waits(nc, cap=_MAX_WAITS):
    """walrus's gen3 codegen rejects >1 sync-wait per instruction; move the
    excess onto NoOps inserted just before (same engine => order kept)."""
    n = 0
    for func in nc.m.functions:
        for bb in func.blocks:
            newlist = []
            for ins in bb.instructions:
                si = getattr(ins, "sync_info", None)
                if si is not None and si.on_wait and len(si.on_wait) > cap:
                    waits = list(si.on_wait)
                    extra, keep = waits[:-cap], waits[-cap:]
                    while extra:
                        chunk, extra = extra[:cap], extra[cap:]
                        nop = mybir.InstNoOp(
                            name=f"{ins.name}-wsplit-{n}", ins=[], outs=[]
                        )
                        nop.engine = ins.engine
                        nop.sync_info = mybir.SyncInfo(on_wait=chunk, on_update=[])
                        newlist.append(nop)
                        n += 1
                    ins.sync_info = mybir.SyncInfo(
                        on_wait=keep, on_update=list(si.on_update)
                    )
                newlist.append(ins)
            bb.instructions = newlist
    return n


def _build_program(r_on_pool):
    nc = bass.Bass()
    Op = mybir.AluOpType

    hidden = nc.declare_dram_parameter("hidden", [NT, H], F32, isOutput=False)
    wrep = nc.declare_dram_parameter("wrep", [P, H], F32, isOutput=False)
    bcol = nc.declare_dram_parameter("bcol", [P, 1], F32, isOutput=False)
    iota_w = nc.declare_dram_parameter("iota_w", [P, W], F32, isOutput=False)
    iota_p = nc.declare_dram_parameter("iota_p", [P, P], F32, isOutput=False)
    iota_m = nc.declare_dram_parameter("iota_m", [P, MAXMEM], F32, isOutput=False)
    bktcol = nc.declare_dram_parameter("bktcol", [P, NCHUNK], F32, isOutput=False)
    offcol = nc.declare_dram_parameter("offcol", [P, NCHUNK], F32, isOutput=False)
    clscol = nc.declare_dram_parameter("clscol", [P, NCHUNK], F32, isOutput=False)
    memcol = nc.declare_dram_parameter("memcol", [P, NCHUNK], F32, isOutput=False)
    fixgid = nc.declare_dram_parameter("fixgid", [P, BS], I32, isOutput=False)
    outs = [
        nc.declare_dram_parameter(f"out{r}", [V], F32, isOutput=True)
        for r in range(BS)
    ]

    NSL = [(0, 512), (512, 1024), (1024, 1536), (1536, W)]

    with tile.TileContext(nc) as tc:
        with (
            tc.tile_pool(name="stream", bufs=4) as stream_tp,
            tc.tile_pool(name="junk", bufs=2) as junk_tp,
            tc.tile_pool(name="rk", bufs=2) as rk_tp,
            tc.tile_pool(name="ak", bufs=3) as ak_tp,
            tc.tile_pool(name="dense", bufs=2) as dense_tp,
            tc.tile_pool(name="psumd", bufs=1, space="PSUM") as psumd_tp,
            tc.tile_pool(name="psumf", bufs=2, space="PSUM") as psumf_tp,
            tc.tile_pool(name="persist", bufs=1) as pers_tp,
        ):
            # ---- one-time loads ----
            wt = pers_tp.tile([P, H], F32, tag="wt")
            nc.sync.dma_start(out=wt[:], in_=wrep[:])
            iw = pers_tp.tile([P, W], F32, tag="iw")
            nc.sync.dma_start(out=iw[:], in_=iota_w[:])
            ip = pers_tp.tile([P, P], F32, tag="ip")
            nc.sync.dma_start(out=ip[:], in_=iota_p[:])
            im = pers_tp.tile([P, MAXMEM], F32, tag="im")
            nc.sync.dma_start(out=im[:], in_=iota_m[:])
            bkt_t = pers_tp.tile([P, NCHUNK], F32, tag="bkt")
            nc.sync.dma_start(out=bkt_t[:], in_=bktcol[:])
            off_t = pers_tp.tile([P, NCHUNK], F32, tag="off")
            nc.sync.dma_start(out=off_t[:], in_=offcol[:])
            cls_t = pers_tp.tile([P, NCHUNK], F32, tag="cls")
            nc.sync.dma_start(out=cls_t[:], in_=clscol[:])
            mem_t = pers_tp.tile([P, NCHUNK], F32, tag="mem")
            nc.sync.dma_start(out=mem_t[:], in_=memcol[:])
            bcol_t = pers_tp.tile([P, 1], F32, tag="bcol")
            nc.sync.dma_start(out=bcol_t[:], in_=bcol[:])
            fg_t = pers_tp.tile([P, BS], I32, tag="fg")
            nc.sync.dma_start(out=fg_t[:], in_=fixgid[:])

            twraw = pers_tp.tile([P, NCHUNK], F32, tag="twraw")
            tw = pers_tp.tile([P, NCHUNK], F32, tag="tw")
            fixv = pers_tp.tile([P, BS], F32, tag="fixv")

            for r in range(BS):
                cols = slice(r * CPR, (r + 1) * CPR)
                # ---- matvec for this row's 8 chunks ----
                for j in range(CPR):
                    k = r * CPR + j
                    x = stream_tp.tile([P, H], F32, tag="x")
                    nc.sync.dma_start(out=x[:], in_=hidden[k * P : (k + 1) * P, :])
                    junk = junk_tp.tile([P, H], F32, tag="junk")
                    nc.vector.scalar_tensor_tensor(
                        out=junk[:], in0=x[:], scalar=1.0, in1=wt[:],
                        op0=Op.mult, op1=Op.mult,
                        accum_out=twraw[:, k : k + 1],
                    )
                nc.vector.tensor_scalar(
                    out=tw[:, cols], in0=twraw[:, cols],
                    scalar1=bcol_t[:, 0:1], scalar2=0.0,
                    op0=Op.add, op1=Op.max,
                )
                # ---- dense assembly + fixup bucketing ----
                psd = psumd_tp.tile([P, W], F32, tag="psd")
                psf = psumf_tp.tile([P, MAXMEM], F32, tag="psf")
                for j in range(CPR):
                    k = r * CPR + j
                    ak = ak_tp.tile([P, P], F32, tag="ak")
                    nc.vector.tensor_scalar(
                        out=ak[:], in0=ip[:],
                        scalar1=bkt_t[:, k : k + 1], scalar2=tw[:, k : k + 1],
                        op0=Op.is_equal, op1=Op.mult,
                    )
                    rk = rk_tp.tile([P, W], F32, tag="rk")
                    eng = nc.gpsimd if r_on_pool else nc.vector
                    eng.tensor_scalar(
                        out=rk[:], in0=iw[:],
                        scalar1=off_t[:, k : k + 1], scalar2=None,
                        op0=Op.is_equal,
                    )
                    for n0, n1 in NSL:
                        nc.tensor.matmul(
                            out=psd[:, n0:n1], lhsT=ak[:], rhs=rk[:, n0:n1],
                            start=(j == 0), stop=(j == CPR - 1),
                        )
                    # fixup: class-bucket the duplicate-class member weights
                    lk = ak_tp.tile([P, P], F32, tag="lk")
                    nc.vector.tensor_scalar(
                        out=lk[:], in0=ip[:],
                        scalar1=cls_t[:, k : k + 1], scalar2=tw[:, k : k + 1],
                        op0=Op.is_equal, op1=Op.mult,
                    )
                    mk = ak_tp.tile([P, MAXMEM], F32, tag="mk")
                    nc.vector.tensor_scalar(
                        out=mk[:], in0=im[:],
                        scalar1=mem_t[:, k : k + 1], scalar2=None,
                        op0=Op.is_equal,
                    )
                    nc.tensor.matmul(
                        out=psf[:], lhsT=lk[:], rhs=mk[:],
                        start=(j == 0), stop=(j == CPR - 1),
                    )
                # class max over member slots -> per-class fixup values
                nc.vector.tensor_reduce(
                    out=fixv[:, r : r + 1], in_=psf[:],
                    axis=mybir.AxisListType.X, op=Op.max,
                )
                # dense tile out of PSUM, then write the row
                dn = dense_tp.tile([P, W], F32, tag="dn")
                nc.any.tensor_copy(out=dn[:], in_=psd[:])
                nc.sync.dma_start(
                    out=outs[r][0 : 127 * W].rearrange("(p f) -> p f", f=W),
                    in_=dn[0:127, :],
                )
                nc.sync.dma_start(
                    out=outs[r][127 * W : V].rearrange("(a f) -> a f", a=1),
                    in_=dn[127:128, 0 : V - 127 * W],
                )
                # fixup scatter: one 128-index indirect DMA (D=1), OOB-padded
                nc.gpsimd.indirect_dma_start(
                    out=outs[r][:].unsqueeze(1),
                    out_offset=IndirectOffsetOnAxis(ap=fg_t[:, r : r + 1], axis=0),
                    in_=fixv[:, r : r + 1],
                    in_offset=None,
                    bounds_check=V - 1,
                    oob_is_err=False,
                )

    _split_excess_waits(nc)
    return nc


_prog_cache = {}


def _get_program(r_on_pool=False):
    key = ("nc", r_on_pool)
    if key not in _prog_cache:
        _prog_cache[key] = _build_program(r_on_pool)
    return _prog_cache[key]


def _make_in_maps(hidden_state, input_ids, w_sparse, b_sparse):
    hs = np.asarray(hidden_state, dtype=np.float32).reshape(B, L, H)
    ids_all = np.asarray(input_ids).astype(np.int64).reshape(B, L)
    w = np.asarray(w_sparse, dtype=np.float32).reshape(H)
    bval = float(np.asarray(b_sparse, dtype=np.float32).reshape(-1)[0])

    wrep = np.ascontiguousarray(np.broadcast_to(w, (P, H)))
    bcol = np.full((P, 1), bval, dtype=np.float32)
    iota_w = np.broadcast_to(np.arange(W, dtype=np.float32), (P, W)).copy()
    iota_p = np.broadcast_to(np.arange(P, dtype=np.float32), (P, P)).copy()
    iota_m = np.broadcast_to(np.arange(MAXMEM, dtype=np.float32), (P, MAXMEM)).copy()

    in_maps = []
    for c in range(NCORES):
        ids = ids_all[c * BS : (c + 1) * BS]                 # (BS, L)
        bkt = np.full((P, NCHUNK), -1.0, np.float32)
        off = np.zeros((P, NCHUNK), np.float32)
        clsc = np.full((P, NCHUNK), -1.0, np.float32)
        memc = np.full((P, NCHUNK), -1.0, np.float32)
        fg = np.full((P, BS), V, np.int32)                   # V => out of bounds
        for r in range(BS):
            row = ids[r]
            vals, counts = np.unique(row, return_counts=True)
            dupset = {int(v) for v, n in zip(vals, counts) if n > 1 and v >= 4}
            dup_list = sorted(dupset)
            assert len(dup_list) <= MAXCLS, f"too many duplicate classes: {len(dup_list)}"
            clsidx = {v: i for i, v in enumerate(dup_list)}
            memcount = {v: 0 for v in dup_list}
            for q, v in enumerate(dup_list):
                fg[q, r] = v
            for l in range(L):
                tid = int(row[l])
                p, j = l % P, l // P
                k = r * CPR + j
                if tid < 4:
                    continue
                if tid in clsidx:
                    clsc[p, k] = clsidx[tid]
                    m = memcount[tid]
                    assert m < MAXMEM, "duplicate class larger than MAXMEM"
                    memc[p, k] = m
                    memcount[tid] = m + 1
                else:
                    bkt[p, k] = tid // W
                    off[p, k] = tid % W
        in_maps.append(
            {
                "hidden": np.ascontiguousarray(
                    hs[c * BS : (c + 1) * BS].reshape(NT, H)
                ),
                "wrep": wrep,
                "bcol": bcol,
                "iota_w": iota_w,
                "iota_p": iota_p,
                "iota_m": iota_m,
                "bktcol": bkt,
                "offcol": off,
                "clscol": clsc,
                "memcol": memc,
                "fixgid": fg,
            }
        )
    return in_maps


def kernel(hidden_state, input_ids, w_sparse, b_sparse, _trace=False,
           _r_on_pool=False):
    nc = _get_program(_r_on_pool)
    in_maps = _make_in_maps(hidden_state, input_ids, w_sparse, b_sparse)
    res = run_bass_kernel_spmd(nc, in_maps, list(range(NCORES)), trace=_trace)
    parts = [
        np.stack([np.asarray(res.results[c][f"out{r}"]) for r in range(BS)])
        for c in range(NCORES)
    ]
    full = np.concatenate(parts, axis=0)
    if _trace:
        kernel.last_exec_time_ns = res.exec_time_ns
        kernel.last_results = res
    return full


# revision 6
# speedup vs baseline: 3.9799x; 3.9799x over previous
"""BGE-M3 sparse-embedding head (matvec + relu + scatter-max into (B, V))
as a Bass/Tile kernel on 8 Trainium2 NeuronCores.

Sharding: data-parallel over batch; each core computes 4 of 32 rows.

Per core:
  1. tw = relu(hidden @ w + b) streamed in 128-token tiles, computed with a
     fused scalar_tensor_tensor (multiply + free-dim sum) on the vector engine.
  2. Output rows are assembled DENSELY in SBUF and written out with plain
     DMAs (no data-dependent scatter DMAs): for each row, two exact one-hot
     factors route every singleton token's weight to its (partition, offset)
     cell of a (128, 1954) tile via PE matmuls:
        A_k[t, p] = (iota128 == id//1954) * tw[t]   (per-partition scalars)
        R_k[t, f] = (iota1954 == id%1954)
        dense += A_k^T @ R_k   (accumulated over the row's 8 token chunks)
     Distinct vocab ids hit distinct cells, so the sums are exact.
  3. Duplicate vocab ids within a row (a handful per row; the class structure
     is a pure function of input_ids, so the host computes it) are excluded
     from the dense path and resolved exactly by a small matmul that buckets
     each class's member weights into one PSUM row, a free-dim reduce_max,
     and ONE 128-index indirect DMA scatter per row (disjoint from the dense
     positions, so no ordering hazards beyond row-level sequencing).
Special tokens 0..3 are never routed, leaving zeros from the dense tile.
"""

import numpy as np

import concourse.bass as bass
import concourse.mybir as mybir
import concourse.tile as tile
from concourse.bass import IndirectOffsetOnAxis
from concourse.bass_utils import run_bass_kernel_spmd

V = 250002
NCORES = 8
B, L, H = 32, 1024, 1024
BS = B // NCORES            # batch rows per core
NT = BS * L                 # tokens per core
P = 128
CPR = L // P                # chunks per row (8)
NCHUNK = NT // P            # chunks per core (32)
W = 1954                    # dense row width per partition (128*1954 >= V)
MAXCLS = P                  # fixup classes per row (<=128)
MAXMEM = 8                  # members per duplicate class
F32 = mybir.dt.float32
BF16 = mybir.dt.bfloat16
I32 = mybir.dt.int32

_MAX_WAITS = 1


def _split_excess_waits(nc, cap=_MAX_WAITS):
    """walrus's gen3 codegen rejects >1 sync-wait per instruction; move the
    excess onto NoOps inserted just before (same engine => order kept)."""
    n = 0
    for func in nc.m.functions:
        for bb in func.blocks:
            newlist = []
            for ins in bb.instructions:
                si = getattr(ins, "sync_info", None)
                if si is not None and si.on_wait and len(si.on_wait) > cap:
                    waits = list(si.on_wait)
                    extra, keep = waits[:-cap], waits[-cap:]
                    while extra:
                        chunk, extra = extra[:cap], extra[cap:]
                        nop = mybir.InstNoOp(
                            name=f"{ins.name}-wsplit-{n}", ins=[], outs=[]
                        )
                        nop.engine = ins.engine
                        nop.sync_info = mybir.SyncInfo(on_wait=chunk, on_update=[])
                        newlist.append(nop)
                        n += 1
                    ins.sync_info = mybir.SyncInfo(
                        on_wait=keep, on_update=list(si.on_update)
                    )
                newlist.append(ins)
            bb.instructions = newlist
    return n


def _build_program(r_on_pool):
    nc = bass.Bass()
    Op = mybir.AluOpType

    hidden = nc.declare_dram_parameter("hidden", [NT, H], F32, isOutput=False)
    wrep = nc.declare_dram_parameter("wrep", [P, H], F32, isOutput=False)
    bcol = nc.declare_dram_parameter("bcol", [P, 1], F32, isOutput=False)
    iota_w = nc.declare_dram_parameter("iota_w", [P, W], F32, isOutput=False)
    iota_p = nc.declare_dram_parameter("iota_p", [P, P], F32, isOutput=False)
    iota_m = nc.declare_dram_parameter("iota_m", [P, MAXMEM], F32, isOutput=False)
    bktcol = nc.declare_dram_parameter("bktcol", [P, NCHUNK], F32, isOutput=False)
    offcol = nc.declare_dram_parameter("offcol", [P, NCHUNK], F32, isOutput=False)
    clscol = nc.declare_dram_parameter("clscol", [P, NCHUNK], F32, isOutput=False)
    memcol = nc.declare_dram_parameter("memcol", [P, NCHUNK], F32, isOutput=False)
    fixgid = nc.declare_dram_parameter("fixgid", [P, BS], I32, isOutput=False)
    outs = [
        nc.declare_dram_parameter(f"out{r}", [V], F32, isOutput=True)
        for r in range(BS)
    ]

    NSL = [(0, 512), (512, 1024), (1024, 1536), (1536, W)]

    with tile.TileContext(nc) as tc:
        with (
            tc.tile_pool(name="stream", bufs=4) as stream_tp,
            tc.tile_pool(name="junk", bufs=2) as junk_tp,
            tc.tile_pool(name="rk", bufs=2) as rk_tp,
            tc.tile_pool(name="ak", bufs=3) as ak_tp,
            tc.tile_pool(name="dense", bufs=2) as dense_tp,
            tc.tile_pool(name="psumd", bufs=1, space="PSUM") as psumd_tp,
            tc.tile_pool(name="psumf", bufs=2, space="PSUM") as psumf_tp,
            tc.tile_pool(name="persist", bufs=1) as pers_tp,
        ):
            # ---- one-time loads ----
            wt = pers_tp.tile([P, H], F32, tag="wt")
            nc.sync.dma_start(out=wt[:], in_=wrep[:])
            iw = pers_tp.tile([P, W], F32, tag="iw")
            nc.sync.dma_start(out=iw[:], in_=iota_w[:])
            ip = pers_tp.tile([P, P], F32, tag="ip")
            nc.sync.dma_start(out=ip[:], in_=iota_p[:])
            im = pers_tp.tile([P, MAXMEM], F32, tag="im")
            nc.sync.dma_start(out=im[:], in_=iota_m[:])
            bkt_t = pers_tp.tile([P, NCHUNK], F32, tag="bkt")
            nc.sync.dma_start(out=bkt_t[:], in_=bktcol[:])
            off_t = pers_tp.tile([P, NCHUNK], F32, tag="off")
            nc.sync.dma_start(out=off_t[:], in_=offcol[:])
            cls_t = pers_tp.tile([P, NCHUNK], F32, tag="cls")
            nc.sync.dma_start(out=cls_t[:], in_=clscol[:])
            mem_t = pers_tp.tile([P, NCHUNK], F32, tag="mem")
            nc.sync.dma_start(out=mem_t[:], in_=memcol[:])
            bcol_t = pers_tp.tile([P, 1], F32, tag="bcol")
            nc.sync.dma_start(out=bcol_t[:], in_=bcol[:])
            fg_t = pers_tp.tile([P, BS], I32, tag="fg")
            nc.sync.dma_start(out=fg_t[:], in_=fixgid[:])

            twraw = pers_tp.tile([P, NCHUNK], F32, tag="twraw")
            tw = pers_tp.tile([P, NCHUNK], F32, tag="tw")
            twbf = pers_tp.tile([P, NCHUNK], BF16, tag="twbf")
            twlo = pers_tp.tile([P, NCHUNK], F32, tag="twlo")
            fixv = pers_tp.tile([P, BS], F32, tag="fixv")

            for r in range(BS):
                cols = slice(r * CPR, (r + 1) * CPR)
                # ---- matvec for this row's 8 chunks ----
                for j in range(CPR):
                    k = r * CPR + j
                    x = stream_tp.tile([P, H], F32, tag="x")
                    nc.sync.dma_start(out=x[:], in_=hidden[k * P : (k + 1) * P, :])
                    junk = junk_tp.tile([P, H], F32, tag="junk")
                    nc.vector.scalar_tensor_tensor(
                        out=junk[:], in0=x[:], scalar=1.0, in1=wt[:],
                        op0=Op.mult, op1=Op.mult,
                        accum_out=twraw[:, k : k + 1],
                    )
                nc.vector.tensor_scalar(
                    out=tw[:, cols], in0=twraw[:, cols],
                    scalar1=bcol_t[:, 0:1], scalar2=0.0,
                    op0=Op.add, op1=Op.max,
                )
                nc.vector.tensor_copy(out=twbf[:, cols], in_=tw[:, cols])
                nc.vector.tensor_tensor(
                    out=twlo[:, cols], in0=tw[:, cols], in1=twbf[:, cols],
                    op=Op.subtract,
                )
                # ---- dense assembly + fixup bucketing ----
                psd = psumd_tp.tile([P, W], F32, tag="psd")
                psf = psumf_tp.tile([P, MAXMEM], F32, tag="psf")
                for j in range(CPR):
                    k = r * CPR + j
                    akh = ak_tp.tile([P, P], BF16, tag="akh")
                    nc.vector.tensor_scalar(
                        out=akh[:], in0=ip[:],
                        scalar1=bkt_t[:, k : k + 1], scalar2=tw[:, k : k + 1],
                        op0=Op.is_equal, op1=Op.mult,
                    )
                    akl = ak_tp.tile([P, P], BF16, tag="akl")
                    nc.vector.tensor_scalar(
                        out=akl[:], in0=ip[:],
                        scalar1=bkt_t[:, k : k + 1], scalar2=twlo[:, k : k + 1],
                        op0=Op.is_equal, op1=Op.mult,
                    )
                    rk = rk_tp.tile([P, W], BF16, tag="rk")
                    nc.vector.tensor_scalar(
                        out=rk[:], in0=iw[:],
                        scalar1=off_t[:, k : k + 1], scalar2=None,
                        op0=Op.is_equal,
                    )
                    for n0, n1 in NSL:
                        nc.tensor.matmul(
                            out=psd[:, n0:n1], lhsT=akh[:], rhs=rk[:, n0:n1],
                            start=(j == 0), stop=False,
                        )
                        nc.tensor.matmul(
                            out=psd[:, n0:n1], lhsT=akl[:], rhs=rk[:, n0:n1],
                            start=False, stop=(j == CPR - 1),
                        )
                    # fixup: class-bucket the duplicate-class member weights
                    lkh = ak_tp.tile([P, P], BF16, tag="lkh")
                    nc.vector.tensor_scalar(
                        out=lkh[:], in0=ip[:],
                        scalar1=cls_t[:, k : k + 1], scalar2=tw[:, k : k + 1],
                        op0=Op.is_equal, op1=Op.mult,
                    )
                    lkl = ak_tp.tile([P, P], BF16, tag="lkl")
                    nc.vector.tensor_scalar(
                        out=lkl[:], in0=ip[:],
                        scalar1=cls_t[:, k : k + 1], scalar2=twlo[:, k : k + 1],
                        op0=Op.is_equal, op1=Op.mult,
                    )
                    mk = ak_tp.tile([P, MAXMEM], BF16, tag="mk")
                    nc.vector.tensor_scalar(
                        out=mk[:], in0=im[:],
                        scalar1=mem_t[:, k : k + 1], scalar2=None,
                        op0=Op.is_equal,
                    )
                    nc.tensor.matmul(
                        out=psf[:], lhsT=lkh[:], rhs=mk[:],
                        start=(j == 0), stop=False,
                    )
                    nc.tensor.matmul(
                        out=psf[:], lhsT=lkl[:], rhs=mk[:],
                        start=False, stop=(j == CPR - 1),
                    )
                # class max over member slots -> per-class fixup values
                nc.vector.tensor_reduce(
                    out=fixv[:, r : r + 1], in_=psf[:],
                    axis=mybir.AxisListType.X, op=Op.max,
                )
                # dense tile out of PSUM, then write the row
                dn = dense_tp.tile([P, W], F32, tag="dn")
                nc.any.tensor_copy(out=dn[:], in_=psd[:])
                nc.sync.dma_start(
                    out=outs[r][0 : 127 * W].rearrange("(p f) -> p f", f=W),
                    in_=dn[0:127, :],
                )
                nc.sync.dma_start(
                    out=outs[r][127 * W : V].rearrange("(a f) -> a f", a=1),
                    in_=dn[127:128, 0 : V - 127 * W],
                )
                # fixup scatter: one 128-index indirect DMA (D=1), OOB-padded
                nc.gpsimd.indirect_dma_start(
                    out=outs[r][:].unsqueeze(1),
                    out_offset=IndirectOffsetOnAxis(ap=fg_t[:, r : r + 1], axis=0),
                    in_=fixv[:, r : r + 1],
                    in_offset=None,
                    bounds_check=V - 1,
                    oob_is_err=False,
                )

    _split_excess_waits(nc)
    return nc


_prog_cache = {}


def _get_program(r_on_pool=False):
    key = ("nc", r_on_pool)
    if key not in _prog_cache:
        _prog_cache[key] = _build_program(r_on_pool)
    return _prog_cache[key]


def _make_in_maps(hidden_state, input_ids, w_sparse, b_sparse):
    hs = np.asarray(hidden_state, dtype=np.float32).reshape(B, L, H)
    ids_all = np.asarray(input_ids).astype(np.int64).reshape(B, L)
    w = np.asarray(w_sparse, dtype=np.float32).reshape(H)
    bval = float(np.asarray(b_sparse, dtype=np.float32).reshape(-1)[0])

    wrep = np.ascontiguousarray(np.broadcast_to(w, (P, H)))
    bcol = np.full((P, 1), bval, dtype=np.float32)
    iota_w = np.broadcast_to(np.arange(W, dtype=np.float32), (P, W)).copy()
    iota_p = np.broadcast_to(np.arange(P, dtype=np.float32), (P, P)).copy()
    iota_m = np.broadcast_to(np.arange(MAXMEM, dtype=np.float32), (P, MAXMEM)).copy()

    in_maps = []
    for c in range(NCORES):
        ids = ids_all[c * BS : (c + 1) * BS]                 # (BS, L)
        bkt = np.full((P, NCHUNK), -1.0, np.float32)
        off = np.zeros((P, NCHUNK), np.float32)
        clsc = np.full((P, NCHUNK), -1.0, np.float32)
        memc = np.full((P, NCHUNK), -1.0, np.float32)
        fg = np.full((P, BS), V, np.int32)                   # V => out of bounds
        for r in range(BS):
            row = ids[r]
            vals, counts = np.unique(row, return_counts=True)
            dupset = {int(v) for v, n in zip(vals, counts) if n > 1 and v >= 4}
            dup_list = sorted(dupset)
            assert len(dup_list) <= MAXCLS, f"too many duplicate classes: {len(dup_list)}"
            clsidx = {v: i for i, v in enumerate(dup_list)}
            memcount = {v: 0 for v in dup_list}
            for q, v in enumerate(dup_list):
                fg[q, r] = v
            for l in range(L):
                tid = int(row[l])
                p, j = l % P, l // P
                k = r * CPR + j
                if tid < 4:
                    continue
                if tid in clsidx:
                    clsc[p, k] = clsidx[tid]
                    m = memcount[tid]
                    assert m < MAXMEM, "duplicate class larger than MAXMEM"
                    memc[p, k] = m
                    memcount[tid] = m + 1
                else:
                    bkt[p, k] = tid // W
                    off[p, k] = tid % W
        in_maps.append(
            {
                "hidden": np.ascontiguousarray(
                    hs[c * BS : (c + 1) * BS].reshape(NT, H)
                ),
                "wrep": wrep,
                "bcol": bcol,
                "iota_w": iota_w,
                "iota_p": iota_p,
                "iota_m": iota_m,
                "bktcol": bkt,
                "offcol": off,
                "clscol": clsc,
                "memcol": memc,
                "fixgid": fg,
            }
        )
    return in_maps


def kernel(hidden_state, input_ids, w_sparse, b_sparse, _trace=False,
           _r_on_pool=False):
    nc = _get_program(_r_on_pool)
    in_maps = _make_in_maps(hidden_state, input_ids, w_sparse, b_sparse)
    res = run_bass_kernel_spmd(nc, in_maps, list(range(NCORES)), trace=_trace)
    parts = [
        np.stack([np.asarray(res.results[c][f"out{r}"]) for r in range(BS)])
        for c in range(NCORES)
    ]
    full = np.concatenate(parts, axis=0)
    if _trace:
        kernel.last_exec_time_ns = res.exec_time_ns
        kernel.last_results = res
    return full


# revision 7
# speedup vs baseline: 4.3143x; 1.0840x over previous
"""BGE-M3 sparse-embedding head (matvec + relu + scatter-max into (B, V))
as a Bass/Tile kernel on 8 Trainium2 NeuronCores.

Sharding: data-parallel over batch; each core computes 4 of 32 rows.

Per core:
  1. tw = relu(hidden @ w + b) streamed in 128-token tiles, computed with a
     fused scalar_tensor_tensor (multiply + free-dim sum) on the vector engine.
  2. Output rows are assembled DENSELY in SBUF and written out with plain
     DMAs (no data-dependent scatter DMAs): for each row, two exact one-hot
     factors route every singleton token's weight to its (partition, offset)
     cell of a (128, 1954) tile via PE matmuls:
        A_k[t, p] = (iota128 == id//1954) * tw[t]   (per-partition scalars)
        R_k[t, f] = (iota1954 == id%1954)
        dense += A_k^T @ R_k   (accumulated over the row's 8 token chunks)
     Distinct vocab ids hit distinct cells, so the sums are exact.
  3. Duplicate vocab ids within a row (a handful per row; the class structure
     is a pure function of input_ids, so the host computes it) are excluded
     from the dense path and resolved exactly by a small matmul that buckets
     each class's member weights into one PSUM row, a free-dim reduce_max,
     and ONE 128-index indirect DMA scatter per row (disjoint from the dense
     positions, so no ordering hazards beyond row-level sequencing).
Special tokens 0..3 are never routed, leaving zeros from the dense tile.
"""

import numpy as np

import concourse.bass as bass
import concourse.mybir as mybir
import concourse.tile as tile
from concourse.bass import IndirectOffsetOnAxis
from concourse.bass_utils import run_bass_kernel_spmd

V = 250002
NCORES = 8
B, L, H = 32, 1024, 1024
BS = B // NCORES            # batch rows per core
NT = BS * L                 # tokens per core
P = 128
CPR = L // P                # chunks per row (8)
NCHUNK = NT // P            # chunks per core (32)
W = 1954                    # dense row width per partition (128*1954 >= V)
MAXCLS = P                  # fixup classes per row (<=128)
MAXMEM = 8                  # members per duplicate class
F32 = mybir.dt.float32
BF16 = mybir.dt.bfloat16
I32 = mybir.dt.int32

_MAX_WAITS = 1


def _split_excess_waits(nc, cap=_MAX_WAITS):
    """walrus's gen3 codegen rejects >1 sync-wait per instruction; move the
    excess onto NoOps inserted just before (same engine => order kept)."""
    n = 0
    for func in nc.m.functions:
        for bb in func.blocks:
            newlist = []
            for ins in bb.instructions:
                si = getattr(ins, "sync_info", None)
                if si is not None and si.on_wait and len(si.on_wait) > cap:
                    waits = list(si.on_wait)
                    extra, keep = waits[:-cap], waits[-cap:]
                    while extra:
                        chunk, extra = extra[:cap], extra[cap:]
                        nop = mybir.InstNoOp(
                            name=f"{ins.name}-wsplit-{n}", ins=[], outs=[]
                        )
                        nop.engine = ins.engine
                        nop.sync_info = mybir.SyncInfo(on_wait=chunk, on_update=[])
                        newlist.append(nop)
                        n += 1
                    ins.sync_info = mybir.SyncInfo(
                        on_wait=keep, on_update=list(si.on_update)
                    )
                newlist.append(ins)
            bb.instructions = newlist
    return n


def _build_program(r_on_pool):
    nc = bass.Bass()
    Op = mybir.AluOpType

    hidden = nc.declare_dram_parameter("hidden", [NT, H], F32, isOutput=False)
    wrep = nc.declare_dram_parameter("wrep", [P, H], F32, isOutput=False)
    bcol = nc.declare_dram_parameter("bcol", [P, 1], F32, isOutput=False)
    iota_w = nc.declare_dram_parameter("iota_w", [P, W], F32, isOutput=False)
    iota_p = nc.declare_dram_parameter("iota_p", [P, P], F32, isOutput=False)
    iota_m = nc.declare_dram_parameter("iota_m", [P, MAXMEM], F32, isOutput=False)
    bktcol = nc.declare_dram_parameter("bktcol", [P, NCHUNK], F32, isOutput=False)
    offcol = nc.declare_dram_parameter("offcol", [P, NCHUNK], F32, isOutput=False)
    clscol = nc.declare_dram_parameter("clscol", [P, NCHUNK], F32, isOutput=False)
    memcol = nc.declare_dram_parameter("memcol", [P, NCHUNK], F32, isOutput=False)
    fixgid = nc.declare_dram_parameter("fixgid", [P, BS], I32, isOutput=False)
    outs = [
        nc.declare_dram_parameter(f"out{r}", [V], F32, isOutput=True)
        for r in range(BS)
    ]

    NSL = [(0, 512), (512, 1024), (1024, 1536), (1536, W)]

    with tile.TileContext(nc) as tc:
        with (
            tc.tile_pool(name="stream", bufs=4) as stream_tp,
            tc.tile_pool(name="junk", bufs=2) as junk_tp,
            tc.tile_pool(name="rk", bufs=2) as rk_tp,
            tc.tile_pool(name="ak", bufs=3) as ak_tp,
            tc.tile_pool(name="dense", bufs=2) as dense_tp,
            tc.tile_pool(name="psumd", bufs=1, space="PSUM") as psumd_tp,
            tc.tile_pool(name="psumf", bufs=2, space="PSUM") as psumf_tp,
            tc.tile_pool(name="persist", bufs=1) as pers_tp,
        ):
            # ---- one-time loads ----
            wt = pers_tp.tile([P, H], F32, tag="wt")
            nc.sync.dma_start(out=wt[:], in_=wrep[:])
            iw = pers_tp.tile([P, W], F32, tag="iw")
            nc.sync.dma_start(out=iw[:], in_=iota_w[:])
            ip = pers_tp.tile([P, P], F32, tag="ip")
            nc.sync.dma_start(out=ip[:], in_=iota_p[:])
            im = pers_tp.tile([P, MAXMEM], F32, tag="im")
            nc.sync.dma_start(out=im[:], in_=iota_m[:])
            bkt_t = pers_tp.tile([P, NCHUNK], F32, tag="bkt")
            nc.sync.dma_start(out=bkt_t[:], in_=bktcol[:])
            off_t = pers_tp.tile([P, NCHUNK], F32, tag="off")
            nc.sync.dma_start(out=off_t[:], in_=offcol[:])
            cls_t = pers_tp.tile([P, NCHUNK], F32, tag="cls")
            nc.sync.dma_start(out=cls_t[:], in_=clscol[:])
            mem_t = pers_tp.tile([P, NCHUNK], F32, tag="mem")
            nc.sync.dma_start(out=mem_t[:], in_=memcol[:])
            bcol_t = pers_tp.tile([P, 1], F32, tag="bcol")
            nc.sync.dma_start(out=bcol_t[:], in_=bcol[:])
            fg_t = pers_tp.tile([P, BS], I32, tag="fg")
            nc.sync.dma_start(out=fg_t[:], in_=fixgid[:])

            twraw = pers_tp.tile([P, NCHUNK], F32, tag="twraw")
            tw = pers_tp.tile([P, NCHUNK], F32, tag="tw")
            twbf = pers_tp.tile([P, NCHUNK], BF16, tag="twbf")
            twlo = pers_tp.tile([P, NCHUNK], F32, tag="twlo")
            fixv = pers_tp.tile([P, BS], F32, tag="fixv")

            for r in range(BS):
                cols = slice(r * CPR, (r + 1) * CPR)
                # ---- matvec for this row's 8 chunks ----
                for j in range(CPR):
                    k = r * CPR + j
                    x = stream_tp.tile([P, H], F32, tag="x")
                    deng = nc.sync if j % 2 == 0 else nc.scalar
                    deng.dma_start(out=x[:], in_=hidden[k * P : (k + 1) * P, :])
                    junk = junk_tp.tile([P, H], F32, tag="junk")
                    nc.vector.scalar_tensor_tensor(
                        out=junk[:], in0=x[:], scalar=1.0, in1=wt[:],
                        op0=Op.mult, op1=Op.mult,
                        accum_out=twraw[:, k : k + 1],
                    )
                nc.vector.tensor_scalar(
                    out=tw[:, cols], in0=twraw[:, cols],
                    scalar1=bcol_t[:, 0:1], scalar2=0.0,
                    op0=Op.add, op1=Op.max,
                )
                nc.vector.tensor_copy(out=twbf[:, cols], in_=tw[:, cols])
                nc.vector.tensor_tensor(
                    out=twlo[:, cols], in0=tw[:, cols], in1=twbf[:, cols],
                    op=Op.subtract,
                )
                # ---- dense assembly + fixup bucketing ----
                psd = psumd_tp.tile([P, W], F32, tag="psd")
                psf = psumf_tp.tile([P, MAXMEM], F32, tag="psf")
                for j in range(CPR):
                    k = r * CPR + j
                    akh = ak_tp.tile([P, P], BF16, tag="akh")
                    nc.vector.tensor_scalar(
                        out=akh[:], in0=ip[:],
                        scalar1=bkt_t[:, k : k + 1], scalar2=tw[:, k : k + 1],
                        op0=Op.is_equal, op1=Op.mult,
                    )
                    akl = ak_tp.tile([P, P], BF16, tag="akl")
                    nc.vector.tensor_scalar(
                        out=akl[:], in0=ip[:],
                        scalar1=bkt_t[:, k : k + 1], scalar2=twlo[:, k : k + 1],
                        op0=Op.is_equal, op1=Op.mult,
                    )
                    rk = rk_tp.tile([P, W], BF16, tag="rk")
                    nc.vector.tensor_scalar(
                        out=rk[:], in0=iw[:],
                        scalar1=off_t[:, k : k + 1], scalar2=None,
                        op0=Op.is_equal,
                    )
                    for n0, n1 in NSL:
                        nc.tensor.matmul(
                            out=psd[:, n0:n1], lhsT=akh[:], rhs=rk[:, n0:n1],
                            start=(j == 0), stop=False,
                        )
                        nc.tensor.matmul(
                            out=psd[:, n0:n1], lhsT=akl[:], rhs=rk[:, n0:n1],
                            start=False, stop=(j == CPR - 1),
                        )
                    # fixup: class-bucket the duplicate-class member weights
                    lkh = ak_tp.tile([P, P], BF16, tag="lkh")
                    nc.vector.tensor_scalar(
                        out=lkh[:], in0=ip[:],
                        scalar1=cls_t[:, k : k + 1], scalar2=tw[:, k : k + 1],
                        op0=Op.is_equal, op1=Op.mult,
                    )
                    lkl = ak_tp.tile([P, P], BF16, tag="lkl")
                    nc.vector.tensor_scalar(
                        out=lkl[:], in0=ip[:],
                        scalar1=cls_t[:, k : k + 1], scalar2=twlo[:, k : k + 1],
                        op0=Op.is_equal, op1=Op.mult,
                    )
                    mk = ak_tp.tile([P, MAXMEM], BF16, tag="mk")
                    nc.vector.tensor_scalar(
                        out=mk[:], in0=im[:],
                        scalar1=mem_t[:, k : k + 1], scalar2=None,
                        op0=Op.is_equal,
                    )
                    nc.tensor.matmul(
                        out=psf[:], lhsT=lkh[:], rhs=mk[:],
                        start=(j == 0), stop=False,
                    )
                    nc.tensor.matmul(
                        out=psf[:], lhsT=lkl[:], rhs=mk[:],
                        start=False, stop=(j == CPR - 1),
                    )
                # class max over member slots -> per-class fixup values
                nc.vector.tensor_reduce(
                    out=fixv[:, r : r + 1], in_=psf[:],
                    axis=mybir.AxisListType.X, op=Op.max,
                )
                # dense tile out of PSUM, then write the row
                dn = dense_tp.tile([P, W], F32, tag="dn")
                nc.any.tensor_copy(out=dn[:], in_=psd[:])
                nc.scalar.dma_start(
                    out=outs[r][0 : 127 * W].rearrange("(p f) -> p f", f=W),
                    in_=dn[0:127, :],
                )
                nc.scalar.dma_start(
                    out=outs[r][127 * W : V].rearrange("(a f) -> a f", a=1),
                    in_=dn[127:128, 0 : V - 127 * W],
                )
                # fixup scatter: one 128-index indirect DMA (D=1), OOB-padded
                nc.gpsimd.indirect_dma_start(
                    out=outs[r][:].unsqueeze(1),
                    out_offset=IndirectOffsetOnAxis(ap=fg_t[:, r : r + 1], axis=0),
                    in_=fixv[:, r : r + 1],
                    in_offset=None,
                    bounds_check=V - 1,
                    oob_is_err=False,
                )

    _split_excess_waits(nc)
    return nc


_prog_cache = {}


def _get_program(r_on_pool=False):
    key = ("nc", r_on_pool)
    if key not in _prog_cache:
        _prog_cache[key] = _build_program(r_on_pool)
    return _prog_cache[key]


def _make_in_maps(hidden_state, input_ids, w_sparse, b_sparse):
    hs = np.asarray(hidden_state, dtype=np.float32).reshape(B, L, H)
    ids_all = np.asarray(input_ids).astype(np.int64).reshape(B, L)
    w = np.asarray(w_sparse, dtype=np.float32).reshape(H)
    bval = float(np.asarray(b_sparse, dtype=np.float32).reshape(-1)[0])

    wrep = np.ascontiguousarray(np.broadcast_to(w, (P, H)))
    bcol = np.full((P, 1), bval, dtype=np.float32)
    iota_w = np.broadcast_to(np.arange(W, dtype=np.float32), (P, W)).copy()
    iota_p = np.broadcast_to(np.arange(P, dtype=np.float32), (P, P)).copy()
    iota_m = np.broadcast_to(np.arange(MAXMEM, dtype=np.float32), (P, MAXMEM)).copy()

    in_maps = []
    for c in range(NCORES):
        ids = ids_all[c * BS : (c + 1) * BS]                 # (BS, L)
        bkt = np.full((P, NCHUNK), -1.0, np.float32)
        off = np.zeros((P, NCHUNK), np.float32)
        clsc = np.full((P, NCHUNK), -1.0, np.float32)
        memc = np.full((P, NCHUNK), -1.0, np.float32)
        fg = np.full((P, BS), V, np.int32)                   # V => out of bounds
        for r in range(BS):
            row = ids[r]
            vals, counts = np.unique(row, return_counts=True)
            dupset = {int(v) for v, n in zip(vals, counts) if n > 1 and v >= 4}
            dup_list = sorted(dupset)
            assert len(dup_list) <= MAXCLS, f"too many duplicate classes: {len(dup_list)}"
            clsidx = {v: i for i, v in enumerate(dup_list)}
            memcount = {v: 0 for v in dup_list}
            for q, v in enumerate(dup_list):
                fg[q, r] = v
            for l in range(L):
                tid = int(row[l])
                p, j = l % P, l // P
                k = r * CPR + j
                if tid < 4:
                    continue
                if tid in clsidx:
                    clsc[p, k] = clsidx[tid]
                    m = memcount[tid]
                    assert m < MAXMEM, "duplicate class larger than MAXMEM"
                    memc[p, k] = m
                    memcount[tid] = m + 1
                else:
                    bkt[p, k] = tid // W
                    off[p, k] = tid % W
        in_maps.append(
            {
                "hidden": np.ascontiguousarray(
                    hs[c * BS : (c + 1) * BS].reshape(NT, H)
                ),
                "wrep": wrep,
                "bcol": bcol,
                "iota_w": iota_w,
                "iota_p": iota_p,
                "iota_m": iota_m,
                "bktcol": bkt,
                "offcol": off,
                "clscol": clsc,
                "memcol": memc,
                "fixgid": fg,
            }
        )
    return in_maps


def kernel(hidden_state, input_ids, w_sparse, b_sparse, _trace=False,
           _r_on_pool=False):
    nc = _get_program(_r_on_pool)
    in_maps = _make_in_maps(hidden_state, input_ids, w_sparse, b_sparse)
    res = run_bass_kernel_spmd(nc, in_maps, list(range(NCORES)), trace=_trace)
    parts = [
        np.stack([np.asarray(res.results[c][f"out{r}"]) for r in range(BS)])
        for c in range(NCORES)
    ]
    full = np.concatenate(parts, axis=0)
    if _trace:
        kernel.last_exec_time_ns = res.exec_time_ns
        kernel.last_results = res
    return full


# revision 8
# speedup vs baseline: 6.5986x; 1.5295x over previous
"""BGE-M3 sparse-embedding head (matvec + relu + scatter-max into (B, V))
as a Bass/Tile kernel on 8 Trainium2 NeuronCores.

Sharding: data-parallel over batch; each core computes 4 of 32 rows.

Per core:
  1. tw = relu(hidden @ w + b) streamed in 128-token tiles, computed with a
     fused scalar_tensor_tensor (multiply + free-dim sum) on the vector engine.
  2. Output rows are assembled DENSELY in SBUF and written out with plain
     DMAs (no data-dependent scatter DMAs): for each row, two exact one-hot
     factors route every singleton token's weight to its (partition, offset)
     cell of a (128, 1954) tile via PE matmuls:
        A_k[t, p] = (iota128 == id//1954) * tw[t]   (per-partition scalars)
        R_k[t, f] = (iota1954 == id%1954)
        dense += A_k^T @ R_k   (accumulated over the row's 8 token chunks)
     Distinct vocab ids hit distinct cells, so the sums are exact.
  3. Duplicate vocab ids within a row (a handful per row; the class structure
     is a pure function of input_ids, so the host computes it) are excluded
     from the dense path and resolved exactly by a small matmul that buckets
     each class's member weights into one PSUM row, a free-dim reduce_max,
     and ONE 128-index indirect DMA scatter per row (disjoint from the dense
     positions, so no ordering hazards beyond row-level sequencing).
Special tokens 0..3 are never routed, leaving zeros from the dense tile.
"""

import numpy as np

import concourse.bass as bass
import concourse.mybir as mybir
import concourse.tile as tile
from concourse.bass import IndirectOffsetOnAxis
from concourse.bass_utils import run_bass_kernel_spmd

V = 250002
NCORES = 8
B, L, H = 32, 1024, 1024
BS = B // NCORES            # batch rows per core
NT = BS * L                 # tokens per core
P = 128
CPR = L // P                # chunks per row (8)
NCHUNK = NT // P            # chunks per core (32)
W = 1954                    # dense row width per partition (128*1954 >= V)
MAXCLS = P                  # fixup classes per row (<=128)
MAXMEM = 8                  # members per duplicate class
F32 = mybir.dt.float32
BF16 = mybir.dt.bfloat16
I32 = mybir.dt.int32

_MAX_WAITS = 1


def _split_excess_waits(nc, cap=_MAX_WAITS):
    """walrus's gen3 codegen rejects >1 sync-wait per instruction; move the
    excess onto NoOps inserted just before (same engine => order kept)."""
    n = 0
    for func in nc.m.functions:
        for bb in func.blocks:
            newlist = []
            for ins in bb.instructions:
                si = getattr(ins, "sync_info", None)
                if si is not None and si.on_wait and len(si.on_wait) > cap:
                    waits = list(si.on_wait)
                    extra, keep = waits[:-cap], waits[-cap:]
                    while extra:
                        chunk, extra = extra[:cap], extra[cap:]
                        nop = mybir.InstNoOp(
                            name=f"{ins.name}-wsplit-{n}", ins=[], outs=[]
                        )
                        nop.engine = ins.engine
                        nop.sync_info = mybir.SyncInfo(on_wait=chunk, on_update=[])
                        newlist.append(nop)
                        n += 1
                    ins.sync_info = mybir.SyncInfo(
                        on_wait=keep, on_update=list(si.on_update)
                    )
                newlist.append(ins)
            bb.instructions = newlist
    return n


def _build_program(r_on_pool):
    nc = bass.Bass()
    Op = mybir.AluOpType

    hidden = nc.declare_dram_parameter("hidden", [NT, H], F32, isOutput=False)
    wrep = nc.declare_dram_parameter("wrep", [P, H], F32, isOutput=False)
    bcol = nc.declare_dram_parameter("bcol", [P, 1], F32, isOutput=False)
    iota_w = nc.declare_dram_parameter("iota_w", [P, W], F32, isOutput=False)
    iota_p = nc.declare_dram_parameter("iota_p", [P, P], F32, isOutput=False)
    iota_m = nc.declare_dram_parameter("iota_m", [P, MAXMEM], F32, isOutput=False)
    bktcol = nc.declare_dram_parameter("bktcol", [P, NCHUNK], F32, isOutput=False)
    offcol = nc.declare_dram_parameter("offcol", [P, NCHUNK], F32, isOutput=False)
    clscol = nc.declare_dram_parameter("clscol", [P, NCHUNK], F32, isOutput=False)
    memcol = nc.declare_dram_parameter("memcol", [P, NCHUNK], F32, isOutput=False)
    fixgid = nc.declare_dram_parameter("fixgid", [P, BS], I32, isOutput=False)
    outs = [
        nc.declare_dram_parameter(f"out{r}", [V], F32, isOutput=True)
        for r in range(BS)
    ]

    NSL = [(0, 512), (512, 1024), (1024, 1536), (1536, W)]

    with tile.TileContext(nc) as tc:
        with (
            tc.tile_pool(name="stream", bufs=4) as stream_tp,
            tc.tile_pool(name="junk", bufs=2) as junk_tp,
            tc.tile_pool(name="rk", bufs=2) as rk_tp,
            tc.tile_pool(name="ak", bufs=3) as ak_tp,
            tc.tile_pool(name="dense", bufs=2) as dense_tp,
            tc.tile_pool(name="psumd", bufs=1, space="PSUM") as psumd_tp,
            tc.tile_pool(name="psumf", bufs=2, space="PSUM") as psumf_tp,
            tc.tile_pool(name="persist", bufs=1) as pers_tp,
        ):
            # ---- one-time loads ----
            wt = pers_tp.tile([P, H], F32, tag="wt")
            nc.sync.dma_start(out=wt[:], in_=wrep[:])
            iw = pers_tp.tile([P, W], F32, tag="iw")
            nc.sync.dma_start(out=iw[:], in_=iota_w[:])
            ip = pers_tp.tile([P, P], F32, tag="ip")
            nc.sync.dma_start(out=ip[:], in_=iota_p[:])
            im = pers_tp.tile([P, MAXMEM], F32, tag="im")
            nc.sync.dma_start(out=im[:], in_=iota_m[:])
            bkt_t = pers_tp.tile([P, NCHUNK], F32, tag="bkt")
            nc.sync.dma_start(out=bkt_t[:], in_=bktcol[:])
            off_t = pers_tp.tile([P, NCHUNK], F32, tag="off")
            nc.sync.dma_start(out=off_t[:], in_=offcol[:])
            cls_t = pers_tp.tile([P, NCHUNK], F32, tag="cls")
            nc.sync.dma_start(out=cls_t[:], in_=clscol[:])
            mem_t = pers_tp.tile([P, NCHUNK], F32, tag="mem")
            nc.sync.dma_start(out=mem_t[:], in_=memcol[:])
            bcol_t = pers_tp.tile([P, 1], F32, tag="bcol")
            nc.sync.dma_start(out=bcol_t[:], in_=bcol[:])
            fg_t = pers_tp.tile([P, BS], I32, tag="fg")
            nc.sync.dma_start(out=fg_t[:], in_=fixgid[:])

            twraw = pers_tp.tile([P, NCHUNK], F32, tag="twraw")
            tw = pers_tp.tile([P, NCHUNK], F32, tag="tw")
            twbf = pers_tp.tile([P, NCHUNK], BF16, tag="twbf")
            twlo = pers_tp.tile([P, NCHUNK], F32, tag="twlo")
            fixv = pers_tp.tile([P, BS], F32, tag="fixv")

            for r in range(BS):
                cols = slice(r * CPR, (r + 1) * CPR)
                # ---- matvec for this row's 8 chunks ----
                for j in range(CPR):
                    k = r * CPR + j
                    x = stream_tp.tile([P, H], F32, tag="x")
                    deng = nc.sync if j % 2 == 0 else nc.scalar
                    deng.dma_start(out=x[:], in_=hidden[k * P : (k + 1) * P, :])
                    junk = junk_tp.tile([P, H], F32, tag="junk")
                    nc.vector.scalar_tensor_tensor(
                        out=junk[:], in0=x[:], scalar=1.0, in1=wt[:],
                        op0=Op.mult, op1=Op.mult,
                        accum_out=twraw[:, k : k + 1],
                    )
                nc.vector.tensor_scalar(
                    out=tw[:, cols], in0=twraw[:, cols],
                    scalar1=bcol_t[:, 0:1], scalar2=0.0,
                    op0=Op.add, op1=Op.max,
                )
                nc.vector.tensor_copy(out=twbf[:, cols], in_=tw[:, cols])
                nc.vector.tensor_tensor(
                    out=twlo[:, cols], in0=tw[:, cols], in1=twbf[:, cols],
                    op=Op.subtract,
                )
                # ---- dense assembly + fixup bucketing ----
                psd = psumd_tp.tile([P, W], F32, tag="psd")
                psf = psumf_tp.tile([P, MAXMEM], F32, tag="psf")
                for j in range(CPR):
                    k = r * CPR + j
                    akh = ak_tp.tile([P, P], BF16, tag="akh")
                    nc.vector.tensor_scalar(
                        out=akh[:], in0=ip[:],
                        scalar1=bkt_t[:, k : k + 1], scalar2=tw[:, k : k + 1],
                        op0=Op.is_equal, op1=Op.mult,
                    )
                    akl = ak_tp.tile([P, P], BF16, tag="akl")
                    nc.vector.tensor_scalar(
                        out=akl[:], in0=ip[:],
                        scalar1=bkt_t[:, k : k + 1], scalar2=twlo[:, k : k + 1],
                        op0=Op.is_equal, op1=Op.mult,
                    )
                    rk = rk_tp.tile([P, W], BF16, tag="rk")
                    nc.vector.tensor_scalar(
                        out=rk[:], in0=iw[:],
                        scalar1=off_t[:, k : k + 1], scalar2=None,
                        op0=Op.is_equal,
                    )
                    for n0, n1 in NSL:
                        nc.tensor.matmul(
                            out=psd[:, n0:n1], lhsT=akh[:], rhs=rk[:, n0:n1],
                            start=(j == 0), stop=False,
                        )
                        nc.tensor.matmul(
                            out=psd[:, n0:n1], lhsT=akl[:], rhs=rk[:, n0:n1],
                            start=False, stop=(j == CPR - 1),
                        )
                    # fixup: class-bucket the duplicate-class member weights
                    lkh = ak_tp.tile([P, P], BF16, tag="lkh")
                    nc.vector.tensor_scalar(
                        out=lkh[:], in0=ip[:],
                        scalar1=cls_t[:, k : k + 1], scalar2=tw[:, k : k + 1],
                        op0=Op.is_equal, op1=Op.mult,
                    )
                    lkl = ak_tp.tile([P, P], BF16, tag="lkl")
                    nc.vector.tensor_scalar(
                        out=lkl[:], in0=ip[:],
                        scalar1=cls_t[:, k : k + 1], scalar2=twlo[:, k : k + 1],
                        op0=Op.is_equal, op1=Op.mult,
                    )
                    mk = ak_tp.tile([P, MAXMEM], BF16, tag="mk")
                    nc.vector.tensor_scalar(
                        out=mk[:], in0=im[:],
                        scalar1=mem_t[:, k : k + 1], scalar2=None,
                        op0=Op.is_equal,
                    )
                    nc.tensor.matmul(
                        out=psf[:], lhsT=lkh[:], rhs=mk[:],
                        start=(j == 0), stop=False,
                    )
                    nc.tensor.matmul(
                        out=psf[:], lhsT=lkl[:], rhs=mk[:],
                        start=False, stop=(j == CPR - 1),
                    )
                # class max over member slots -> per-class fixup values
                nc.vector.tensor_reduce(
                    out=fixv[:, r : r + 1], in_=psf[:],
                    axis=mybir.AxisListType.X, op=Op.max,
                )
                # dense tile out of PSUM, then write the row
                dn = dense_tp.tile([P, W], F32, tag="dn")
                nc.any.tensor_copy(out=dn[:], in_=psd[:])
                for si, (p0, p1) in enumerate(((0, 32), (32, 64), (64, 96), (96, 127))):
                    oeng = nc.scalar if si % 2 == 0 else nc.sync
                    oeng.dma_start(
                        out=outs[r][p0 * W : p1 * W].rearrange("(p f) -> p f", f=W),
                        in_=dn[p0:p1, :],
                    )
                nc.sync.dma_start(
                    out=outs[r][127 * W : V].rearrange("(a f) -> a f", a=1),
                    in_=dn[127:128, 0 : V - 127 * W],
                )
                # fixup scatter: one 128-index indirect DMA (D=1), OOB-padded
                nc.gpsimd.indirect_dma_start(
                    out=outs[r][:].unsqueeze(1),
                    out_offset=IndirectOffsetOnAxis(ap=fg_t[:, r : r + 1], axis=0),
                    in_=fixv[:, r : r + 1],
                    in_offset=None,
                    bounds_check=V - 1,
                    oob_is_err=False,
                )

    _split_excess_waits(nc)
    return nc


_prog_cache = {}


def _get_program(r_on_pool=False):
    key = ("nc", r_on_pool)
    if key not in _prog_cache:
        _prog_cache[key] = _build_program(r_on_pool)
    return _prog_cache[key]


def _make_in_maps(hidden_state, input_ids, w_sparse, b_sparse):
    hs = np.asarray(hidden_state, dtype=np.float32).reshape(B, L, H)
    ids_all = np.asarray(input_ids).astype(np.int64).reshape(B, L)
    w = np.asarray(w_sparse, dtype=np.float32).reshape(H)
    bval = float(np.asarray(b_sparse, dtype=np.float32).reshape(-1)[0])

    wrep = np.ascontiguousarray(np.broadcast_to(w, (P, H)))
    bcol = np.full((P, 1), bval, dtype=np.float32)
    iota_w = np.broadcast_to(np.arange(W, dtype=np.float32), (P, W)).copy()
    iota_p = np.broadcast_to(np.arange(P, dtype=np.float32), (P, P)).copy()
    iota_m = np.broadcast_to(np.arange(MAXMEM, dtype=np.float32), (P, MAXMEM)).copy()

    in_maps = []
    for c in range(NCORES):
        ids = ids_all[c * BS : (c + 1) * BS]                 # (BS, L)
        bkt = np.full((P, NCHUNK), -1.0, np.float32)
        off = np.zeros((P, NCHUNK), np.float32)
        clsc = np.full((P, NCHUNK), -1.0, np.float32)
        memc = np.full((P, NCHUNK), -1.0, np.float32)
        fg = np.full((P, BS), V, np.int32)                   # V => out of bounds
        for r in range(BS):
            row = ids[r]
            vals, counts = np.unique(row, return_counts=True)
            dupset = {int(v) for v, n in zip(vals, counts) if n > 1 and v >= 4}
            dup_list = sorted(dupset)
            assert len(dup_list) <= MAXCLS, f"too many duplicate classes: {len(dup_list)}"
            clsidx = {v: i for i, v in enumerate(dup_list)}
            memcount = {v: 0 for v in dup_list}
            for q, v in enumerate(dup_list):
                fg[q, r] = v
            for l in range(L):
                tid = int(row[l])
                p, j = l % P, l // P
                k = r * CPR + j
                if tid < 4:
                    continue
                if tid in clsidx:
                    clsc[p, k] = clsidx[tid]
                    m = memcount[tid]
                    assert m < MAXMEM, "duplicate class larger than MAXMEM"
                    memc[p, k] = m
                    memcount[tid] = m + 1
                else:
                    bkt[p, k] = tid // W
                    off[p, k] = tid % W
        in_maps.append(
            {
                "hidden": np.ascontiguousarray(
                    hs[c * BS : (c + 1) * BS].reshape(NT, H)
                ),
                "wrep": wrep,
                "bcol": bcol,
                "iota_w": iota_w,
                "iota_p": iota_p,
                "iota_m": iota_m,
                "bktcol": bkt,
                "offcol": off,
                "clscol": clsc,
                "memcol": memc,
                "fixgid": fg,
            }
        )
    return in_maps


def kernel(hidden_state, input_ids, w_sparse, b_sparse, _trace=False,
           _r_on_pool=False):
    nc = _get_program(_r_on_pool)
    in_maps = _make_in_maps(hidden_state, input_ids, w_sparse, b_sparse)
    res = run_bass_kernel_spmd(nc, in_maps, list(range(NCORES)), trace=_trace)
    parts = [
        np.stack([np.asarray(res.results[c][f"out{r}"]) for r in range(BS)])
        for c in range(NCORES)
    ]
    full = np.concatenate(parts, axis=0)
    if _trace:
        kernel.last_exec_time_ns = res.exec_time_ns
        kernel.last_results = res
    return full
